# revision 1
# baseline (speedup 1.0000x reference)
"""Distributed GCN (GCNRecommender) Trainium2 Bass kernel.

kernel(**inputs) -> np.ndarray [100000, 32] float32.

Strategy (8 NeuronCores, SPMD single program):
- Nodes sharded across cores; per-core node->(window, slot) packing is
  degree-balanced so one program fits all cores (fixed chunk budgets).
- Per conv: the "gather table" holds per-node rows (pre-multiplied by
  W and dinv where algebraically possible, bf16, 256B rows); each core
  dma_gathers its edges' source rows (int16 idx, 4 sub-tables), builds
  one-hot S matrices on DVE (is_equal vs iota), and segment-sums via
  TensorE matmuls accumulated in PSUM windows.
- Epilogue per window: + own-row (self loop), *dinv, +bias, LayerNorm,
  ELU, then next-layer table rows = (h*dinv)@W_next via PE transpose +
  matmul, spilled to DRAM; AllGather collective replicates shards.
- Final MLP head computed per window; output assembled on host.
"""
import os
import sys
for _p in ("/opt/trn_rl_repo",):
    if _p not in sys.path:
        sys.path.insert(0, _p)

import numpy as np
import ml_dtypes
import time

import concourse.bass as bass
import concourse.bacc as bacc
import concourse.mybir as mybir
import concourse.tile as tile
from concourse.masks import make_identity
import concourse.bass2jax as bass2jax
from concourse.bass2jax import _bass_exec_p, install_neuronx_cc_hook
import jax
from jax.sharding import Mesh, PartitionSpec, NamedSharding
from jax.experimental.shard_map import shard_map

BF16 = mybir.dt.bfloat16
F32 = mybir.dt.float32
I16 = mybir.dt.int16
AF = mybir.ActivationFunctionType
OP = mybir.AluOpType

NCORES = 8
CHUNK = 128
CALL_CHUNKS = 8
SBATCH = 8
EPB = 4
EPS = 1e-5
N_NODES = 100000

def pack_windows(deg_r, nwin, caps):
    """Pack dsts (rows of deg_r: [nd, 4] per-range edge counts) into nwin
    windows with <=128 dsts per window and per-range capacity caps[w, r].
    Greedy: process dsts in decreasing total degree, place into the
    feasible window with max remaining total capacity.
    Returns win[nd], slot[nd] (partition within window)."""
    nd = deg_r.shape[0]
    order = np.argsort(-deg_r.sum(1), kind="stable")
    rem = caps.astype(np.int64).copy()          # [nwin, 4]
    cnt = np.zeros(nwin, np.int32)              # dsts per window
    win = np.full(nd, -1, np.int32)
    slot = np.zeros(nd, np.int32)
    # vectorized-ish greedy: for each dst pick argmax of remaining total
    # among feasible windows
    rem_tot = rem.sum(1)
    for d in order:
        need = deg_r[d]
        feas = (rem >= need).all(1) & (cnt < 128)
        if not feas.any():
            return None, None
        w = int(np.argmax(np.where(feas, rem_tot, -1)))
        win[d] = w
        slot[d] = cnt[w]
        cnt[w] += 1
        rem[w] -= need
        rem_tot[w] = rem[w].sum()
    return win, slot


def prep(edge_index, n_nodes, nranges=4, sweep_w=16):
    """Build all per-core metadata + index arrays. Returns a dict."""
    N = n_nodes
    NS = N // NCORES
    NWIN = NS // CHUNK + 2          # +2 spare windows (hole slots)
    SLOTS = NWIN * CHUNK            # row slots per core shard
    RS = 2 * SLOTS                  # sub-table rows (2 cores per range)
    src = np.asarray(edge_index[0], dtype=np.int64)
    dst = np.asarray(edge_index[1], dtype=np.int64)
    E = src.shape[0]

    deg = np.bincount(dst, minlength=N).astype(np.float32) + 1.0
    dinv = (1.0 / np.sqrt(deg)).astype(np.float32)

    # self loops are NOT materialized as edges: the kernel adds the node's
    # own table row during the epilogue (own-shard resident in SBUF)
    src_all = src
    dst_all = dst

    core_of = dst_all // NS

    # ---- per-(core, local dst, src-range) degree table -------------------
    # r of an edge is determined by src ROW-ID, which depends on the
    # permutation of the src's core... circular! Break it: define range by
    # SRC's core pair: r = (src // NS) // 2 -> the row-id range
    # [25000r, 25000(r+1)) holds exactly cores 2r, 2r+1 regardless of the
    # within-core permutation. So r(edge) = src_core // 2: permutation-free.
    r_all = (src_all // NS) // 2
    assert nranges == 4

    # per-core packing
    perm_rows = np.zeros(N, np.int64)  # node -> global row id
    packs = []
    for c in range(NCORES):
        m = core_of == c
        ed = (dst_all[m] - c * NS).astype(np.int64)
        er = r_all[m]
        nd = NS
        deg_r = np.zeros((nd, 4), np.int64)
        np.add.at(deg_r, (ed, er), 1)
        packs.append(deg_r)

    # capacities: shared across cores. per-range load per core:
    loads = np.array([p.sum(0) for p in packs])  # [8, 4]
    maxload = loads.max(0)                        # [4]
    caps = np.zeros((NWIN, 4), np.int64)
    for r in range(4):
        # chunk-granular: total chunks = ceil((maxload + slack) / CHUNK),
        # spread evenly over windows
        total = int(maxload[r])
        slack = max(2 * CHUNK, int(0.02 * total))
        nch = -(-(total + slack) // CHUNK)
        base = np.full(NWIN, nch // NWIN, np.int64)
        base[: nch - int(base.sum())] += 1
        caps[:, r] = base * CHUNK
    # feasibility loop: caps must be final before any pack
    ok = False
    for attempt in range(8):
        wins, slots = [], []
        ok = True
        for c in range(NCORES):
            w, s = pack_windows(packs[c], NWIN, caps)
            if w is None:
                ok = False
                break
            wins.append(w)
            slots.append(s)
        if ok:
            break
        # widen: add a chunk to the tightest range on the lowest-cap windows
        ratio = maxload / caps.sum(0)
        r_bad = int(np.argmax(ratio))
        add = max(4, NWIN // 16)
        worst = np.argsort(caps[:, r_bad])[:add]
        caps[worst, r_bad] += CHUNK
    assert ok, "packing failed"

    for c in range(NCORES):
        node_ids = c * NS + np.arange(NS)
        perm_rows[node_ids] = c * SLOTS + wins[c] * CHUNK + slots[c]

    # ---- per-core edge streams -------------------------------------------
    # ordered by (r, window) [sweeps handled at kernel build from window id]
    # each (w, r) group padded to caps[w, r]
    npos = int(caps.sum())  # positions per core per conv (same all cores)
    g_idx = np.zeros((NCORES, npos), np.int16)      # sub-table-local row idx
    g_dstloc = np.full((NCORES, npos), -1, np.float32)  # window-local slot or -1
    pos_meta = []  # list of (w, r) per chunk — same for all cores
    # group ordering: (sweep, r, w) so gather calls (fixed r) span
    # contiguous positions
    nsweep = -(-NWIN // sweep_w)
    off = 0
    group_off = {}
    for s in range(nsweep):
        ws = range(s * sweep_w, min((s + 1) * sweep_w, NWIN))
        for r in range(4):
            for w in ws:
                group_off[(w, r)] = off
                for k in range(int(caps[w, r]) // CHUNK):
                    pos_meta.append((w, r))
                off += int(caps[w, r])
    assert off == npos

    for c in range(NCORES):
        m = core_of == c
        es, ed, er = src_all[m], (dst_all[m] - c * NS), r_all[m]
        srow = perm_rows[es]            # global row id of src
        sloc = (srow - er * RS).astype(np.int64)
        assert (sloc >= 0).all() and (sloc < RS).all()
        w_e = wins[c][ed]
        p_e = slots[c][ed]
        # order edges by (w, r) groups
        okey = w_e.astype(np.int64) * 4 + er
        order = np.argsort(okey, kind="stable")
        es_o, sloc_o, w_o, r_o, p_o = (es[order], sloc[order], w_e[order],
                                       er[order], p_e[order])
        # place into padded stream
        gcounts = np.bincount((w_o * 4 + r_o).astype(np.int64), minlength=NWIN * 4)
        ptr = 0
        for w in range(NWIN):
            for r in range(4):
                g = int(gcounts[w * 4 + r])
                o = group_off[(w, r)]
                g_idx[c, o:o + g] = sloc_o[ptr:ptr + g].astype(np.int16)
                g_dstloc[c, o:o + g] = p_o[ptr:ptr + g].astype(np.float32)
                ptr += g
        assert ptr == es_o.shape[0]

    return dict(
        NS=NS, RS=RS, NWIN=NWIN, SLOTS=SLOTS, npos=npos, caps=caps,
        pos_meta=pos_meta, group_off=group_off, sweep_w=sweep_w,
        nsweep=nsweep,
        perm_rows=perm_rows, wins=wins, slots=slots,
        dinv=dinv, g_idx=g_idx, g_dstloc=g_dstloc,
    )




def _chunks_by_segment(meta):
    """Yield (sweep, r, [chunk indices]) in global chunk order."""
    pos_meta = meta["pos_meta"]
    sweep_w = meta["sweep_w"]
    segs = []
    cur = None
    for k, (w, r) in enumerate(pos_meta):
        s = w // sweep_w
        if cur is None or (s, r) != cur[0]:
            cur = ((s, r), [])
            segs.append(cur)
        cur[1].append(k)
    return [(s, r, ks) for (s, r), ks in segs]


def build(meta, cfg):
    """cfg: dict with F_TBL=[64,128,64], FOUT=[64,128,64] etc."""
    NWIN, SLOTS, RS = meta["NWIN"], meta["SLOTS"], meta["RS"]
    npos, nsweep, sweep_w = meta["npos"], meta["nsweep"], meta["sweep_w"]
    nchunk = npos // CHUNK
    segs = _chunks_by_segment(meta)
    ln_g = cfg.get("ln_nontrivial", [False] * 4)

    DBG_NOGATHER = DBG_NOS = DBG_NOMM = DBG_NOZERO = 0
    nc = bacc.Bacc(None, target_bir_lowering=False, num_swdge_queues=4)

    # ---- dram tensors ---------------------------------------------------
    table1 = nc.dram_tensor("table1", [8 * SLOTS, 128], BF16, kind="ExternalInput")
    gidx = nc.dram_tensor("gidx", [128, npos // 16], I16, kind="ExternalInput")
    dstloc_d = nc.dram_tensor("dstloc", [128, nchunk], BF16, kind="ExternalInput")
    iota_d = nc.dram_tensor("iota", [128, 128], BF16, kind="ExternalInput")
    dinvu_d = nc.dram_tensor("dinvu", [128, NWIN], F32, kind="ExternalInput")
    b_d = [nc.dram_tensor(f"bias{i}", [128, f], F32, kind="ExternalInput")
           for i, f in enumerate([64, 128, 64, 32, 32])]
    g_d = [nc.dram_tensor(f"gam{i}", [128, f], F32, kind="ExternalInput")
           if ln_g[i] else None for i, f in enumerate([64, 128, 64, 32])]
    be_d = [nc.dram_tensor(f"bet{i}", [128, f], F32, kind="ExternalInput")
            if ln_g[i] else None for i, f in enumerate([64, 128, 64, 32])]
    w2_d = nc.dram_tensor("w2b", [64, 128], BF16, kind="ExternalInput")
    w3_d = nc.dram_tensor("w3b", [128, 64], BF16, kind="ExternalInput")
    lw1_d = nc.dram_tensor("lw1b", [64, 32], BF16, kind="ExternalInput")
    lw2_d = nc.dram_tensor("lw2b", [32, 32], BF16, kind="ExternalInput")
    out_y = nc.dram_tensor("out_y", [SLOTS, 32], F32, kind="ExternalOutput")
    DBGOUT = 0
    dbg_h = (nc.dram_tensor("dbg_h", [SLOTS, 128], F32, kind="ExternalOutput")
             if DBGOUT else None)

    own1_d = nc.dram_tensor("own1", [SLOTS, 128], BF16, kind="ExternalInput")
    bounce2 = nc.dram_tensor("bounce2", [SLOTS, 128], BF16)
    table2 = nc.dram_tensor("table2", [8 * SLOTS, 128], BF16, addr_space="Shared")
    bounce3 = nc.dram_tensor("bounce3", [SLOTS, 128], BF16)
    table3 = nc.dram_tensor("table3", [8 * SLOTS, 128], BF16, addr_space="Shared")

    from contextlib import ExitStack
    with tile.TileContext(nc) as tc, ExitStack() as ctx:
        cpool = ctx.enter_context(tc.tile_pool(name="const", bufs=1))
        mpool = ctx.enter_context(tc.tile_pool(name="msg", bufs=6))
        spool = ctx.enter_context(tc.tile_pool(name="sb", bufs=2))
        epool = ctx.enter_context(tc.tile_pool(name="epi", bufs=2))
        pp_sweep = ctx.enter_context(tc.tile_pool(name="psw", bufs=2, space="PSUM"))
        pp_aux = ctx.enter_context(tc.tile_pool(name="paux", bufs=1, space="PSUM"))
        pp_mm = ctx.enter_context(tc.tile_pool(name="pmm", bufs=1, space="PSUM"))
        pp_f1 = ctx.enter_context(tc.tile_pool(name="pf1", bufs=1, space="PSUM"))
        pp_f2 = ctx.enter_context(tc.tile_pool(name="pf2", bufs=1, space="PSUM"))

        # ---- constants ---------------------------------------------------
        dstloc = cpool.tile([128, nchunk], BF16)
        nc.sync.dma_start(out=dstloc[:], in_=dstloc_d[:, :])
        iota = cpool.tile([128, 128], BF16)
        nc.sync.dma_start(out=iota[:], in_=iota_d[:, :])
        dinvu = cpool.tile([128, NWIN], F32)
        nc.sync.dma_start(out=dinvu[:], in_=dinvu_d[:, :])
        biases = []
        for i, f in enumerate([64, 128, 64, 32, 32]):
            tb = cpool.tile([128, f], F32, name=f"biasB{i}")
            nc.sync.dma_start(out=tb[:], in_=b_d[i][:, :])
            biases.append(tb)
        gammas, betas = [], []
        for i, f in enumerate([64, 128, 64, 32]):
            if ln_g[i]:
                tb = cpool.tile([128, f], F32, name=f"gamB{i}")
                nc.sync.dma_start(out=tb[:], in_=g_d[i][:, :])
                gammas.append(tb)
                tb2 = cpool.tile([128, f], F32, name=f"betB{i}")
                nc.sync.dma_start(out=tb2[:], in_=be_d[i][:, :])
                betas.append(tb2)
            else:
                gammas.append(None)
                betas.append(None)
        w2 = cpool.tile([64, 128], BF16)
        nc.sync.dma_start(out=w2[:], in_=w2_d[:, :])
        w3 = cpool.tile([128, 64], BF16)
        nc.sync.dma_start(out=w3[:], in_=w3_d[:, :])
        lw1 = cpool.tile([64, 32], BF16)
        nc.sync.dma_start(out=lw1[:], in_=lw1_d[:, :])
        lw2 = cpool.tile([32, 32], BF16)
        nc.sync.dma_start(out=lw2[:], in_=lw2_d[:, :])
        ident = cpool.tile([128, 128], BF16)
        make_identity(nc, ident[:])
        zw = cpool.tile([1, 128], BF16)
        nc.vector.memset(zw[:], 0.0)
        zr = cpool.tile([1, 512], BF16)
        nc.vector.memset(zr[:], 0.0)
        eps_col = cpool.tile([128, 1], F32)
        nc.vector.memset(eps_col[:], EPS)

        def ln_elu_batch(ps_ap, B, F, li, dinv_b, out_bf, hd_out,
                         own_ap=None):
            """LN+ELU over a [128, B, F] psum slice.
            dinv_b: [128, B] AP or None (pre-scale by dinv).
            own_ap: [128, B, F] bf16 self-loop rows to add pre-scale.
            Returns h tile. If hd_out: also return h*dinv bf16 tile.
            out_bf: elu output directly in bf16 (no hd)."""
            bias = biases[li]
            xa = epool.tile([128, B, F], F32, name="xa")
            if own_ap is not None:
                nc.vector.tensor_tensor(
                    out=xa[:], in0=ps_ap, in1=own_ap, op=OP.add)
                src0 = xa[:]
            else:
                src0 = ps_ap
            if dinv_b is not None:
                nc.vector.tensor_tensor(
                    out=xa[:], in0=src0,
                    in1=dinv_b[:, :, None].to_broadcast([128, B, F]),
                    op=OP.mult)
                nc.vector.tensor_tensor(
                    out=xa[:], in0=xa[:],
                    in1=bias[:, None, :].to_broadcast([128, B, F]),
                    op=OP.add)
            else:
                nc.vector.tensor_tensor(
                    out=xa[:], in0=src0,
                    in1=bias[:, None, :].to_broadcast([128, B, F]),
                    op=OP.add)
            xh = None
            msum = epool.tile([128, B], F32, name="msum")
            nc.vector.tensor_reduce(out=msum[:], in_=xa[:],
                                    axis=mybir.AxisListType.X, op=OP.add)
            mu = epool.tile([128, B], F32, name="mu")
            nc.vector.tensor_scalar_mul(mu[:], msum[:], 1.0 / F)
            xc = epool.tile([128, B, F], F32, name="xc")
            nc.vector.tensor_tensor(
                out=xc[:], in0=xa[:],
                in1=mu[:, :, None].to_broadcast([128, B, F]),
                op=OP.subtract)
            sq = epool.tile([128, B, F], F32, name="sq")
            nc.vector.tensor_tensor(out=sq[:], in0=xc[:], in1=xc[:], op=OP.mult)
            vsum = epool.tile([128, B], F32, name="vsum")
            nc.vector.tensor_reduce(out=vsum[:], in_=sq[:],
                                    axis=mybir.AxisListType.X, op=OP.add)
            sd = epool.tile([128, B], F32, name="sd")
            nc.scalar.activation(out=sd[:], in_=vsum[:], func=AF.Sqrt,
                                 scale=1.0 / F, bias=eps_col[:, :1])
            rstd = epool.tile([128, B], F32, name="rstd")
            nc.vector.reciprocal(rstd[:], sd[:])
            if xh is None:
                xh = epool.tile([128, B, F], F32, name="xh")
                nc.vector.tensor_tensor(
                    out=xh[:], in0=xc[:],
                    in1=rstd[:, :, None].to_broadcast([128, B, F]),
                    op=OP.mult)
            if gammas[li] is not None:
                nc.vector.tensor_tensor(
                    out=xh[:], in0=xh[:],
                    in1=gammas[li][:, None, :].to_broadcast([128, B, F]),
                    op=OP.mult)
                nc.vector.tensor_tensor(
                    out=xh[:], in0=xh[:],
                    in1=betas[li][:, None, :].to_broadcast([128, B, F]),
                    op=OP.add)
            # ELU = (max(x,0)-1) + exp(min(x,0))
            mm_ = epool.tile([128, B, F], F32, name="elum")
            nc.vector.tensor_scalar_min(mm_[:], xh[:], 0.0)
            ee = epool.tile([128, B, F], F32, name="elue")
            nc.scalar.activation(out=ee[:], in_=mm_[:], func=AF.Exp)
            rl = epool.tile([128, B, F], F32, name="elur")
            nc.vector.tensor_scalar(rl[:], xh[:], 0.0, -1.0, OP.max, OP.add)
            h = epool.tile([128, B, F], BF16 if out_bf else F32, name="eluh")
            nc.vector.tensor_tensor(out=h[:], in0=rl[:], in1=ee[:], op=OP.add)
            hd = None
            if hd_out:
                hd = epool.tile([128, B, F], BF16, name="hd")
                nc.vector.tensor_tensor(
                    out=hd[:], in0=h[:],
                    in1=dinv_b[:, :, None].to_broadcast([128, B, F]),
                    op=OP.mult)
            return h, hd

        def conv(li, table_t, F, wnext, fnext, bounce_t):
            """One GCN conv layer. li: 0/1/2. F: table feature width.
            wnext/fnext: weight tile + out width for next table (or None
            for conv3 -> final layers)."""
            # own-shard rows resident for the self-loop contribution:
            # own[p, w, :] = table[CORE*SLOTS + w*128 + p]. Loaded via the
            # partition-id-relative slice: each core reads ITS block.
            own = cpool.tile([128, NWIN, 128], BF16, name="own")
            for w in range(NWIN):
                nc.sync.dma_start(
                    out=own[:, w, :F],
                    in_=own_src_t[li][w * 128:(w + 1) * 128, :F])
            seg_i = 0
            for s in range(nsweep):
                w0 = s * sweep_w
                nw = min(sweep_w, NWIN - w0)
                swt = pp_sweep.tile([128, nw * F], F32, name="sweep")
                # zero-clear via K=1 matmuls (512-f32 spans)
                if not DBG_NOZERO:
                    for b0 in range(0, nw * F, 512):
                        span = min(512, nw * F - b0)
                        nc.tensor.matmul(out=swt[:, b0:b0 + span], lhsT=zw[:1, :],
                                         rhs=zr[:1, :span], start=True, stop=False,
                                         skip_group_check=True)
                # chunks of this sweep, per r segment
                while seg_i < len(segs) and segs[seg_i][0] == s:
                    _, r, ks = segs[seg_i]
                    seg_i += 1
                    for c0 in range(0, len(ks), CALL_CHUNKS):
                        kk = ks[c0:c0 + CALL_CHUNKS]
                        ncall = len(kk)
                        p0 = kk[0] * CHUNK
                        it = mpool.tile([128, CALL_CHUNKS * 8], I16, name="idx")
                        nc.sync.dma_start(
                            out=it[:, :ncall * 8],
                            in_=gidx[:, p0 // 16:(p0 + ncall * CHUNK) // 16])
                        qn = call_counter[0] % 4
                        call_counter[0] += 1
                        msg = mpool.tile([128, CALL_CHUNKS, 128], BF16, name="msg")
                        if DBG_NOGATHER:
                            nc.vector.memset(msg[:, :ncall, :], 0.0)
                        else:
                            nc.gpsimd.dma_gather(
                                out_ap=msg[:, :ncall, :],
                                in_ap=table_t[r * RS:(r + 1) * RS, :],
                                idxs_ap=it[:, :ncall * 8],
                                num_idxs=ncall * CHUNK,
                                num_idxs_reg=ncall * CHUNK,
                                elem_size=128,
                                queue_num=qn,
                            )
                        for b0 in range(0, ncall, SBATCH):
                            nb = min(SBATCH, ncall - b0)
                            St = spool.tile([128, SBATCH, 128], BF16, name="S")
                            k0 = kk[b0]
                            if DBG_NOS:
                                nc.vector.memset(St[:, :nb, :], 0.0)
                            elif True:
                                nc.vector.tensor_tensor(
                                out=St[:, :nb, :],
                                in0=dstloc[:, k0:k0 + nb, None].to_broadcast(
                                    [128, nb, 128]),
                                in1=iota[:, None, :].to_broadcast([128, nb, 128]),
                                op=OP.is_equal)
                            for j in range(nb):
                                if DBG_NOMM:
                                    continue
                                k = kk[b0 + j]
                                w = meta["pos_meta"][k][0]
                                uo = (w - w0) * F
                                nc.tensor.matmul(
                                    out=swt[:, uo:uo + F],
                                    lhsT=St[:, j, :],
                                    rhs=msg[:, b0 + j, :F],
                                    start=False, stop=False,
                                    skip_group_check=True)
                # epilogue for this sweep
                if int(__import__("os").environ.get("NOEPI", "0")):
                    continue
                for u0 in range(0, nw, EPB):
                    B = min(EPB, nw - u0)
                    dv = dinvu[:, w0 + u0:w0 + u0 + B]
                    own_sl = own[:, w0 + u0:w0 + u0 + B, :F]
                    if wnext is not None:
                        h, hd = ln_elu_batch(swt[:, u0 * F:(u0 + B) * F].rearrange(
                            "p (b f) -> p b f", b=B), B, F, li, dv,
                            out_bf=False, hd_out=True, own_ap=own_sl)
                        if dbg_h is not None and li == 0:
                            for u in range(B):
                                ug2 = w0 + u0 + u
                                nc.sync.dma_start(
                                    out=dbg_h[ug2 * 128:(ug2 + 1) * 128, :F],
                                    in_=h[:, u, :])
                        for u in range(B):
                            ug = w0 + u0 + u
                            aux = pp_aux.tile([128, 128], BF16, name="aux")
                            nc.tensor.transpose(out=aux[:F, :], in_=hd[:, u, :],
                                                identity=ident[:])
                            hdT = spool.tile([F, 128], BF16, name="hdT")
                            nc.scalar.copy(out=hdT[:], in_=aux[:F, :])
                            mmp = pp_mm.tile([128, 128], F32, name="mmp")
                            nc.tensor.matmul(out=mmp[:, :fnext], lhsT=hdT[:],
                                             rhs=wnext[:], start=True, stop=True,
                                             skip_group_check=True)
                            spl = spool.tile([128, 128], BF16, name="spl")
                            nc.scalar.copy(out=spl[:, :fnext], in_=mmp[:, :fnext])
                            if fnext < 128:
                                nc.vector.memset(spl[:, fnext:], 0.0)
                            nc.sync.dma_start(
                                out=bounce_t[ug * 128:(ug + 1) * 128, :],
                                in_=spl[:, :])
                    else:
                        # conv3: final layers
                        h, _ = ln_elu_batch(swt[:, u0 * F:(u0 + B) * F].rearrange(
                            "p (b f) -> p b f", b=B), B, F, li, dv,
                            out_bf=True, hd_out=False, own_ap=own_sl)
                        f1 = pp_f1.tile([128, EPB * 32], F32, name="f1")
                        for u in range(B):
                            aux = pp_aux.tile([128, 128], BF16, name="aux")
                            nc.tensor.transpose(out=aux[:64, :], in_=h[:, u, :],
                                                identity=ident[:])
                            h3T = spool.tile([64, 128], BF16, name="hdT")
                            nc.scalar.copy(out=h3T[:], in_=aux[:64, :])
                            nc.tensor.matmul(out=f1[:, u * 32:(u + 1) * 32],
                                             lhsT=h3T[:], rhs=lw1[:],
                                             start=True, stop=True,
                                             skip_group_check=True)
                        y1, _ = ln_elu_batch(f1[:, :B * 32].rearrange(
                            "p (b f) -> p b f", b=B), B, 32, 3, None,
                            out_bf=True, hd_out=False)
                        for u in range(B):
                            ug = w0 + u0 + u
                            aux = pp_aux.tile([128, 128], BF16, name="aux")
                            nc.tensor.transpose(out=aux[:32, :], in_=y1[:, u, :],
                                                identity=ident[:])
                            y1T = spool.tile([32, 128], BF16, name="y1T")
                            nc.scalar.copy(out=y1T[:], in_=aux[:32, :])
                            f2 = pp_f2.tile([128, 32], F32, name="f2")
                            nc.tensor.matmul(out=f2[:], lhsT=y1T[:], rhs=lw2[:],
                                             start=True, stop=True,
                                             skip_group_check=True)
                            ys = epool.tile([128, 32], F32, name="ys")
                            nc.vector.tensor_tensor(
                                out=ys[:], in0=f2[:],
                                in1=biases[4][:, :],
                                op=OP.add)
                            nc.sync.dma_start(
                                out=out_y[ug * 128:(ug + 1) * 128, :],
                                in_=ys[:])

        stage = 3
        call_counter = [0]
        own_src_t = {0: own1_d, 1: bounce2, 2: bounce3}
        conv(0, table1, 64, w2, 128, bounce2)
        if stage >= 2:
            nc.gpsimd.collective_compute(
                "AllGather", OP.bypass, replica_groups=[list(range(8))],
                ins=[bounce2[:, :]], outs=[table2[:, :]])
            conv(1, table2, 128, w3, 64, bounce3)
        if stage >= 3:
            nc.gpsimd.collective_compute(
                "AllGather", OP.bypass, replica_groups=[list(range(8))],
                ins=[bounce3[:, :]], outs=[table3[:, :]])
            conv(2, table3, 64, None, None, None)
        if stage < 3:
            # dummy writes so out_y is produced
            zo = cpool.tile([128, 32], F32)
            nc.vector.memset(zo[:], 0.0)
            for w in range(NWIN):
                nc.sync.dma_start(out=out_y[w * 128:(w + 1) * 128, :], in_=zo[:])

    nc.finalize()
    return nc


def make_inputs(meta, x, W1, b1, W2, b2, W3, b3, lw1, lb1, lw2, lb2,
                gs, bes):
    """Per-core input arrays. gs/bes: [g1,g2,g3,g4], [be1..be4]."""
    N = x.shape[0]
    NS, SLOTS, NWIN = meta["NS"], meta["SLOTS"], meta["NWIN"]
    perm = meta["perm_rows"]
    dinv = meta["dinv"]

    xs1 = (x.astype(np.float32) @ W1.astype(np.float32)) * dinv[:, None]
    t1 = np.zeros((8 * SLOTS, 128), ml_dtypes.bfloat16)
    t1[perm, :64] = xs1.astype(ml_dtypes.bfloat16)

    iota = np.tile(np.arange(128, dtype=np.float32), (128, 1)).astype(
        ml_dtypes.bfloat16)

    ln_nontrivial = [not (np.abs(g - 1).max() < 1e-12 and
                          np.abs(b).max() < 1e-12)
                     for g, b in zip(gs, bes)]

    in_maps = []
    for c in range(8):
        # dinv per (partition, window): node of slot (w,p)
        dinvu = np.zeros((128, NWIN), np.float32)
        node_ids = c * NS + np.arange(NS)
        rows = perm[node_ids] - c * SLOTS
        dinvu[rows % 128, rows // 128] = dinv[node_ids]
        gidx = meta["g_idx"][c]
        gidx_w = np.tile(gidx.reshape(-1, 16).T, (8, 1)).copy()
        dstloc = meta["g_dstloc"][c].reshape(-1, 128).T.astype(
            ml_dtypes.bfloat16).copy()
        d = dict(
            table1=t1, own1=t1[c * SLOTS:(c + 1) * SLOTS].copy(),
            gidx=gidx_w, dstloc=dstloc, iota=iota,
            dinvu=dinvu,
            bias0=np.tile(b1.reshape(1, -1), (128, 1)).astype(np.float32),
            bias1=np.tile(b2.reshape(1, -1), (128, 1)).astype(np.float32),
            bias2=np.tile(b3.reshape(1, -1), (128, 1)).astype(np.float32),
            bias3=np.tile(lb1.reshape(1, -1), (128, 1)).astype(np.float32),
            bias4=np.tile(lb2.reshape(1, -1), (128, 1)).astype(np.float32),
            w2b=W2.astype(ml_dtypes.bfloat16),
            w3b=W3.astype(ml_dtypes.bfloat16),
            lw1b=lw1.astype(ml_dtypes.bfloat16),
            lw2b=lw2.astype(ml_dtypes.bfloat16),
        )
        for i in range(4):
            if ln_nontrivial[i]:
                d[f"gam{i}"] = np.tile(gs[i].reshape(1, -1), (128, 1)).astype(np.float32)
                d[f"bet{i}"] = np.tile(bes[i].reshape(1, -1), (128, 1)).astype(np.float32)
        in_maps.append(d)
    return in_maps, ln_nontrivial


def unshard_output(meta, results):
    """results: list of 8 dicts with out_y [SLOTS, 32]. Returns [N, 32]."""
    NS, SLOTS = meta["NS"], meta["SLOTS"]
    N = 8 * NS
    out = np.zeros((N, 32), np.float32)
    for c in range(8):
        rows = meta["perm_rows"][c * NS:(c + 1) * NS] - c * SLOTS
        out[c * NS:(c + 1) * NS] = results[c]["out_y"][rows]
    return out


class BassRunner:
    def __init__(self, nc, n_cores=8):
        install_neuronx_cc_hook()
        self.nc = nc
        self.n_cores = n_cores
        part_name = (nc.partition_id_tensor.name
                     if nc.partition_id_tensor else None)
        in_names, out_names, out_avals, zero_outs = [], [], [], []
        for alloc in nc.m.functions[0].allocations:
            if not isinstance(alloc, mybir.MemoryLocationSet):
                continue
            name = alloc.memorylocations[0].name
            if alloc.kind == "ExternalInput":
                if name != part_name:
                    in_names.append(name)
            elif alloc.kind == "ExternalOutput":
                shape = tuple(alloc.tensor_shape)
                dtype = mybir.dt.np(alloc.dtype)
                out_names.append(name)
                out_avals.append(jax.core.ShapedArray(shape, dtype))
                zero_outs.append(np.zeros(shape, dtype))
        self.in_names = list(in_names)
        self.out_names = out_names
        self.zero_outs = zero_outs
        n_params = len(in_names)
        n_outs = len(out_avals)
        all_in_names = in_names + out_names
        if part_name is not None:
            all_in_names = all_in_names + [part_name]
        donate = tuple(range(n_params, n_params + n_outs))
        self.n_params = n_params

        def _body(*args):
            operands = list(args)
            if part_name is not None:
                operands.append(bass2jax.partition_id_tensor())
            outs = _bass_exec_p.bind(
                *operands, out_avals=tuple(out_avals),
                in_names=tuple(all_in_names), out_names=tuple(out_names),
                lowering_input_output_aliases=(),
                sim_require_finite=True, sim_require_nnan=True, nc=nc)
            return tuple(outs)

        devices = jax.devices()[:n_cores]
        self.mesh = Mesh(np.asarray(devices), ("core",))
        in_specs = (PartitionSpec("core"),) * (n_params + n_outs)
        out_specs = (PartitionSpec("core"),) * len(out_names)
        self.fn = jax.jit(
            shard_map(_body, mesh=self.mesh, in_specs=in_specs,
                      out_specs=out_specs, check_rep=False),
            donate_argnums=donate, keep_unused=True)
        self.sharding = NamedSharding(self.mesh, PartitionSpec("core"))

    def put_inputs(self, in_maps):
        concat = [np.concatenate([np.asarray(in_maps[c][n])
                                  for c in range(self.n_cores)], axis=0)
                  for n in self.in_names]
        self.in_dev = [jax.device_put(a, self.sharding) for a in concat]

    def _zeros_dev(self):
        return [jax.device_put(
            np.zeros((self.n_cores * z.shape[0], *z.shape[1:]), z.dtype),
            self.sharding) for z in self.zero_outs]

    def run(self):
        outs = self.fn(*self.in_dev, *self._zeros_dev())
        jax.block_until_ready(outs)
        return outs

    def run_timed(self, iters=3):
        """Returns (best_seconds, outs)."""
        zeros = [self._zeros_dev() for _ in range(iters)]
        best = float("inf")
        outs = None
        for i in range(iters):
            t0 = time.perf_counter()
            outs = self.fn(*self.in_dev, *zeros[i])
            jax.block_until_ready(outs)
            dt = time.perf_counter() - t0
            best = min(best, dt)
        return best, outs

    def results(self, outs):
        res = []
        for c in range(self.n_cores):
            d = {}
            for i, n in enumerate(self.out_names):
                arr = np.asarray(outs[i])
                per = arr.shape[0] // self.n_cores
                d[n] = arr[c * per:(c + 1) * per]
            res.append(d)
        return res


_CACHE = {}


def kernel(**inputs):
    x = np.asarray(inputs["x"], np.float32)
    ei = np.asarray(inputs["edge_index"], np.int64)
    gs = [np.asarray(inputs[k], np.float32) for k in ("g1", "g2", "g3", "g4")]
    bes = [np.asarray(inputs[k], np.float32) for k in ("be1", "be2", "be3", "be4")]
    meta = prep(ei, N_NODES, sweep_w=8)
    in_maps, ln_nt = make_inputs(
        meta, x, inputs["W1"], inputs["b1"], inputs["W2"], inputs["b2"],
        inputs["W3"], inputs["b3"], inputs["lw1"], inputs["lb1"],
        inputs["lw2"], inputs["lb2"], gs, bes)
    key = (meta["npos"], tuple(ln_nt))
    if key not in _CACHE:
        nc = build(meta, dict(ln_nontrivial=ln_nt + [False]))
        r = BassRunner(nc, NCORES)
        _CACHE[key] = r
    r = _CACHE[key]
    r.put_inputs(in_maps)
    outs = r.run()
    res = r.results(outs)
    out = unshard_output(meta, res)
    # expose for test harness timing
    kernel._last_runner = r
    kernel._last_meta = meta
    return out



# revision 18
# speedup vs baseline: 1.9591x; 1.9591x over previous
"""Distributed GCN (GCNRecommender) Trainium2 Bass kernel.

kernel(**inputs) -> np.ndarray [100000, 32] float32.

Strategy (8 NeuronCores, SPMD single program):
- Nodes sharded across cores; per-core node->(window, slot) packing is
  degree-balanced so one program fits all cores (fixed chunk budgets).
- Per conv: the "gather table" holds per-node rows (pre-multiplied by
  W and dinv where algebraically possible, bf16, 256B rows); each core
  dma_gathers its edges' source rows (int16 idx, 4 sub-tables), builds
  one-hot S matrices on DVE (is_equal vs iota), and segment-sums via
  TensorE matmuls accumulated in PSUM windows.
- Epilogue per window: + own-row (self loop), *dinv, +bias, LayerNorm,
  ELU, then next-layer table rows = (h*dinv)@W_next via PE transpose +
  matmul, spilled to DRAM; AllGather collective replicates shards.
- Final MLP head computed per window; output assembled on host.
"""
import os
import sys
for _p in ("/opt/trn_rl_repo",):
    if _p not in sys.path:
        sys.path.insert(0, _p)

import numpy as np
import ml_dtypes
import time

import concourse.bass as bass
import concourse.bacc as bacc
import concourse.mybir as mybir
import concourse.tile as tile
from concourse.masks import make_identity
import concourse.bass2jax as bass2jax
from concourse.bass2jax import _bass_exec_p, install_neuronx_cc_hook
import jax
from jax.sharding import Mesh, PartitionSpec, NamedSharding
from jax.experimental.shard_map import shard_map

BF16 = mybir.dt.bfloat16
F32 = mybir.dt.float32
I16 = mybir.dt.int16
AF = mybir.ActivationFunctionType
OP = mybir.AluOpType

NCORES = 8
CHUNK = 128
SBATCH = 8
EPB = 4
EPS = 1e-5
N_NODES = 100000

def pack_windows(deg_r, nwin, caps):
    """Pack dsts (rows of deg_r: [nd, 4] per-range edge counts) into nwin
    windows with <=128 dsts per window and per-range capacity caps[w, r].
    Greedy: process dsts in decreasing total degree, place into the
    feasible window with max remaining total capacity.
    Returns win[nd], slot[nd] (partition within window)."""
    nd = deg_r.shape[0]
    order = np.argsort(-deg_r.sum(1), kind="stable")
    rem = caps.astype(np.int64).copy()          # [nwin, 4]
    cnt = np.zeros(nwin, np.int32)              # dsts per window
    win = np.full(nd, -1, np.int32)
    slot = np.zeros(nd, np.int32)
    # vectorized-ish greedy: for each dst pick argmax of remaining total
    # among feasible windows
    rem_tot = rem.sum(1)
    for d in order:
        need = deg_r[d]
        feas = (rem >= need).all(1) & (cnt < 128)
        if not feas.any():
            return None, None
        w = int(np.argmax(np.where(feas, rem_tot, -1)))
        win[d] = w
        slot[d] = cnt[w]
        cnt[w] += 1
        rem[w] -= need
        rem_tot[w] = rem[w].sum()
    return win, slot


def prep(edge_index, n_nodes, nranges=4, sweep_w=16):
    """Build all per-core metadata + index arrays. Returns a dict."""
    N = n_nodes
    NS = N // NCORES
    NWIN = NS // CHUNK + 2          # +2 spare windows (hole slots)
    SLOTS = NWIN * CHUNK            # row slots per core shard
    RS = 2 * SLOTS                  # sub-table rows (2 cores per range)
    src = np.asarray(edge_index[0], dtype=np.int64)
    dst = np.asarray(edge_index[1], dtype=np.int64)
    E = src.shape[0]

    deg = np.bincount(dst, minlength=N).astype(np.float32) + 1.0
    dinv = (1.0 / np.sqrt(deg)).astype(np.float32)

    # self loops are NOT materialized as edges: the kernel adds the node's
    # own table row during the epilogue (own-shard resident in SBUF)
    src_all = src
    dst_all = dst

    core_of = dst_all // NS

    # ---- per-(core, local dst, src-range) degree table -------------------
    # r of an edge is determined by src ROW-ID, which depends on the
    # permutation of the src's core... circular! Break it: define range by
    # SRC's core pair: r = (src // NS) // 2 -> the row-id range
    # [25000r, 25000(r+1)) holds exactly cores 2r, 2r+1 regardless of the
    # within-core permutation. So r(edge) = src_core // 2: permutation-free.
    r_all = (src_all // NS) // 2
    assert nranges == 4

    # per-core packing
    perm_rows = np.zeros(N, np.int64)  # node -> global row id
    packs = []
    for c in range(NCORES):
        m = core_of == c
        ed = (dst_all[m] - c * NS).astype(np.int64)
        er = r_all[m]
        nd = NS
        deg_r = np.zeros((nd, 4), np.int64)
        np.add.at(deg_r, (ed, er), 1)
        packs.append(deg_r)

    # capacities: shared across cores. per-range load per core:
    loads = np.array([p.sum(0) for p in packs])  # [8, 4]
    maxload = loads.max(0)                        # [4]
    caps = np.zeros((NWIN, 4), np.int64)
    for r in range(4):
        # chunk-granular: total chunks = ceil((maxload + slack) / CHUNK),
        # spread evenly over windows
        total = int(maxload[r])
        slack = max(2 * CHUNK, int(0.02 * total))
        nch = -(-(total + slack) // CHUNK)
        base = np.full(NWIN, nch // NWIN, np.int64)
        base[: nch - int(base.sum())] += 1
        caps[:, r] = base * CHUNK
    # feasibility loop: caps must be final before any pack
    ok = False
    for attempt in range(8):
        wins, slots = [], []
        ok = True
        for c in range(NCORES):
            w, s = pack_windows(packs[c], NWIN, caps)
            if w is None:
                ok = False
                break
            wins.append(w)
            slots.append(s)
        if ok:
            break
        # widen: add a chunk to the tightest range on the lowest-cap windows
        ratio = maxload / caps.sum(0)
        r_bad = int(np.argmax(ratio))
        add = max(4, NWIN // 16)
        worst = np.argsort(caps[:, r_bad])[:add]
        caps[worst, r_bad] += CHUNK
    assert ok, "packing failed"

    for c in range(NCORES):
        node_ids = c * NS + np.arange(NS)
        perm_rows[node_ids] = c * SLOTS + wins[c] * CHUNK + slots[c]

    # ---- per-core edge streams -------------------------------------------
    # ordered by (r, window) [sweeps handled at kernel build from window id]
    # each (w, r) group padded to caps[w, r]
    npos = int(caps.sum())  # positions per core per conv (same all cores)
    g_idx = np.zeros((NCORES, npos), np.int16)      # sub-table-local row idx
    g_dstloc = np.full((NCORES, npos), -1, np.float32)  # window-local slot or -1
    pos_meta = []  # list of (w, r) per chunk — same for all cores
    # group ordering: (sweep, r, w) so gather calls (fixed r) span
    # contiguous positions
    nsweep = -(-NWIN // sweep_w)
    off = 0
    group_off = {}
    for s in range(nsweep):
        ws = range(s * sweep_w, min((s + 1) * sweep_w, NWIN))
        for r in range(4):
            for w in ws:
                group_off[(w, r)] = off
                for k in range(int(caps[w, r]) // CHUNK):
                    pos_meta.append((w, r))
                off += int(caps[w, r])
    assert off == npos

    for c in range(NCORES):
        m = core_of == c
        es, ed, er = src_all[m], (dst_all[m] - c * NS), r_all[m]
        srow = perm_rows[es]            # global row id of src
        sloc = (srow - er * RS).astype(np.int64)
        assert (sloc >= 0).all() and (sloc < RS).all()
        w_e = wins[c][ed]
        p_e = slots[c][ed]
        # order edges by (w, r) groups; within a group sort by src row so
        # gather descriptors hit ascending HBM addresses (page locality)
        okey = (w_e.astype(np.int64) * 4 + er) * RS + sloc
        order = np.argsort(okey, kind="stable")
        es_o, sloc_o, w_o, r_o, p_o = (es[order], sloc[order], w_e[order],
                                       er[order], p_e[order])
        # place into padded stream
        gcounts = np.bincount((w_o * 4 + r_o).astype(np.int64), minlength=NWIN * 4)
        ptr = 0
        for w in range(NWIN):
            for r in range(4):
                g = int(gcounts[w * 4 + r])
                o = group_off[(w, r)]
                g_idx[c, o:o + g] = sloc_o[ptr:ptr + g].astype(np.int16)
                g_dstloc[c, o:o + g] = p_o[ptr:ptr + g].astype(np.float32)
                ptr += g
        assert ptr == es_o.shape[0]

    return dict(
        NS=NS, RS=RS, NWIN=NWIN, SLOTS=SLOTS, npos=npos, caps=caps,
        pos_meta=pos_meta, group_off=group_off, sweep_w=sweep_w,
        nsweep=nsweep,
        perm_rows=perm_rows, wins=wins, slots=slots,
        dinv=dinv, g_idx=g_idx, g_dstloc=g_dstloc,
    )




def _chunks_by_segment(meta):
    """Yield (sweep, r, [chunk indices]) in global chunk order."""
    pos_meta = meta["pos_meta"]
    sweep_w = meta["sweep_w"]
    segs = []
    cur = None
    for k, (w, r) in enumerate(pos_meta):
        s = w // sweep_w
        if cur is None or (s, r) != cur[0]:
            cur = ((s, r), [])
            segs.append(cur)
        cur[1].append(k)
    return [(s, r, ks) for (s, r), ks in segs]


def build(meta, cfg):
    """cfg: dict with F_TBL=[64,128,64], FOUT=[64,128,64] etc."""
    NWIN, SLOTS, RS = meta["NWIN"], meta["SLOTS"], meta["RS"]
    npos, nsweep, sweep_w = meta["npos"], meta["nsweep"], meta["sweep_w"]
    nchunk = npos // CHUNK
    segs = _chunks_by_segment(meta)
    ln_g = cfg.get("ln_nontrivial", [False] * 4)

    _env = os.environ.get
    CALL_CHUNKS = int(_env("CALL_CHUNKS", "8"))
    NQUEUES = int(_env("NQUEUES", "4"))
    DBG_NOGATHER = int(_env("DBG_NOGATHER", "0"))
    DBG_NOS = int(_env("DBG_NOS", "0"))
    DBG_NOMM = int(_env("DBG_NOMM", "0"))
    DBG_NOZERO = int(_env("DBG_NOZERO", "0"))
    DBG_NOCOLL = int(_env("DBG_NOCOLL", "0"))
    DBG_STAGE = int(_env("DBG_STAGE", "3"))
    REPEAT = int(_env("REPEAT", "1"))
    DMASCRATCH = int(_env("DMASCRATCH", str(max(16384, CALL_CHUNKS * 128 * 16))))
    nc = bacc.Bacc(None, target_bir_lowering=False, num_swdge_queues=NQUEUES,
                   dynamic_dma_scratch_size=DMASCRATCH)

    # ---- dram tensors ---------------------------------------------------
    table1 = nc.dram_tensor("table1", [8 * SLOTS, 128], BF16, kind="ExternalInput")
    gidx = nc.dram_tensor("gidx", [128, npos // 16], I16, kind="ExternalInput")
    dstloc_d = nc.dram_tensor("dstloc", [128, nchunk], BF16, kind="ExternalInput")
    iota_d = nc.dram_tensor("iota", [128, 128], BF16, kind="ExternalInput")
    dinvu_d = nc.dram_tensor("dinvu", [128, NWIN], F32, kind="ExternalInput")
    b_d = [nc.dram_tensor(f"bias{i}", [128, f], F32, kind="ExternalInput")
           for i, f in enumerate([64, 128, 64, 32, 32])]
    g_d = [nc.dram_tensor(f"gam{i}", [128, f], F32, kind="ExternalInput")
           if ln_g[i] else None for i, f in enumerate([64, 128, 64, 32])]
    be_d = [nc.dram_tensor(f"bet{i}", [128, f], F32, kind="ExternalInput")
            if ln_g[i] else None for i, f in enumerate([64, 128, 64, 32])]
    w2_d = nc.dram_tensor("w2b", [64, 128], BF16, kind="ExternalInput")
    w3_d = nc.dram_tensor("w3b", [128, 64], BF16, kind="ExternalInput")
    lw1_d = nc.dram_tensor("lw1b", [64, 32], BF16, kind="ExternalInput")
    lw2_d = nc.dram_tensor("lw2b", [32, 32], BF16, kind="ExternalInput")
    out_y = nc.dram_tensor("out_y", [SLOTS, 32], F32, kind="ExternalOutput")
    DBGOUT = 0
    dbg_h = (nc.dram_tensor("dbg_h", [SLOTS, 128], F32, kind="ExternalOutput")
             if DBGOUT else None)

    own1_d = nc.dram_tensor("own1", [SLOTS, 128], BF16, kind="ExternalInput")
    bounce2 = nc.dram_tensor("bounce2", [SLOTS, 128], BF16)
    table2 = nc.dram_tensor("table2", [8 * SLOTS, 128], BF16, addr_space="Shared")
    bounce3 = nc.dram_tensor("bounce3", [SLOTS, 128], BF16)
    table3 = nc.dram_tensor("table3", [8 * SLOTS, 128], BF16, addr_space="Shared")

    from contextlib import ExitStack
    with tile.TileContext(nc) as tc, ExitStack() as ctx:
        cpool = ctx.enter_context(tc.tile_pool(name="const", bufs=1))
        opool = ctx.enter_context(tc.tile_pool(name="own", bufs=2))
        mpool = ctx.enter_context(tc.tile_pool(name="msg", bufs=6))
        spool = ctx.enter_context(tc.tile_pool(name="sb", bufs=2))
        epool = ctx.enter_context(tc.tile_pool(name="epi", bufs=2))
        pp_sweep = ctx.enter_context(tc.tile_pool(name="psw", bufs=2, space="PSUM"))
        pp_aux = ctx.enter_context(tc.tile_pool(name="paux", bufs=1, space="PSUM"))
        pp_mm = ctx.enter_context(tc.tile_pool(name="pmm", bufs=1, space="PSUM"))
        pp_f1 = ctx.enter_context(tc.tile_pool(name="pf1", bufs=1, space="PSUM"))
        pp_f2 = ctx.enter_context(tc.tile_pool(name="pf2", bufs=1, space="PSUM"))

        # ---- constants ---------------------------------------------------
        dstloc = cpool.tile([128, nchunk], BF16)
        nc.sync.dma_start(out=dstloc[:], in_=dstloc_d[:, :])
        iota = cpool.tile([128, 128], BF16)
        nc.sync.dma_start(out=iota[:], in_=iota_d[:, :])
        dinvu = cpool.tile([128, NWIN], F32)
        nc.sync.dma_start(out=dinvu[:], in_=dinvu_d[:, :])
        biases = []
        for i, f in enumerate([64, 128, 64, 32, 32]):
            tb = cpool.tile([128, f], F32, name=f"biasB{i}")
            nc.sync.dma_start(out=tb[:], in_=b_d[i][:, :])
            biases.append(tb)
        gammas, betas = [], []
        for i, f in enumerate([64, 128, 64, 32]):
            if ln_g[i]:
                tb = cpool.tile([128, f], F32, name=f"gamB{i}")
                nc.sync.dma_start(out=tb[:], in_=g_d[i][:, :])
                gammas.append(tb)
                tb2 = cpool.tile([128, f], F32, name=f"betB{i}")
                nc.sync.dma_start(out=tb2[:], in_=be_d[i][:, :])
                betas.append(tb2)
            else:
                gammas.append(None)
                betas.append(None)
        w2 = cpool.tile([64, 128], BF16)
        nc.sync.dma_start(out=w2[:], in_=w2_d[:, :])
        w3 = cpool.tile([128, 64], BF16)
        nc.sync.dma_start(out=w3[:], in_=w3_d[:, :])
        lw1 = cpool.tile([64, 32], BF16)
        nc.sync.dma_start(out=lw1[:], in_=lw1_d[:, :])
        lw2 = cpool.tile([32, 32], BF16)
        nc.sync.dma_start(out=lw2[:], in_=lw2_d[:, :])
        ident = cpool.tile([128, 128], BF16)
        make_identity(nc, ident[:])
        zw = cpool.tile([1, 128], BF16)
        nc.vector.memset(zw[:], 0.0)
        zr = cpool.tile([1, 512], BF16)
        nc.vector.memset(zr[:], 0.0)
        eps_col = cpool.tile([128, 1], F32)
        nc.vector.memset(eps_col[:], EPS)

        def ln_elu_batch(ps_ap, B, F, li, dinv_b, out_bf, hd_out,
                         own_ap=None):
            """LN+ELU over a [128, B, F] psum slice.
            dinv_b: [128, B] AP or None (pre-scale by dinv).
            own_ap: [128, B, F] bf16 self-loop rows to add pre-scale.
            Returns h tile. If hd_out: also return h*dinv bf16 tile.
            out_bf: elu output directly in bf16 (no hd)."""
            bias = biases[li]
            xa = epool.tile([128, B, F], F32, name="xa")
            if own_ap is not None:
                nc.vector.tensor_tensor(
                    out=xa[:], in0=ps_ap, in1=own_ap, op=OP.add)
                src0 = xa[:]
            else:
                src0 = ps_ap
            if dinv_b is not None:
                nc.vector.tensor_tensor(
                    out=xa[:], in0=src0,
                    in1=dinv_b[:, :, None].to_broadcast([128, B, F]),
                    op=OP.mult)
                nc.vector.tensor_tensor(
                    out=xa[:], in0=xa[:],
                    in1=bias[:, None, :].to_broadcast([128, B, F]),
                    op=OP.add)
            else:
                nc.vector.tensor_tensor(
                    out=xa[:], in0=src0,
                    in1=bias[:, None, :].to_broadcast([128, B, F]),
                    op=OP.add)
            xh = None
            msum = epool.tile([128, B], F32, name="msum")
            nc.vector.tensor_reduce(out=msum[:], in_=xa[:],
                                    axis=mybir.AxisListType.X, op=OP.add)
            mu = epool.tile([128, B], F32, name="mu")
            nc.vector.tensor_scalar_mul(mu[:], msum[:], 1.0 / F)
            xc = epool.tile([128, B, F], F32, name="xc")
            nc.vector.tensor_tensor(
                out=xc[:], in0=xa[:],
                in1=mu[:, :, None].to_broadcast([128, B, F]),
                op=OP.subtract)
            sq = epool.tile([128, B, F], F32, name="sq")
            nc.vector.tensor_tensor(out=sq[:], in0=xc[:], in1=xc[:], op=OP.mult)
            vsum = epool.tile([128, B], F32, name="vsum")
            nc.vector.tensor_reduce(out=vsum[:], in_=sq[:],
                                    axis=mybir.AxisListType.X, op=OP.add)
            sd = epool.tile([128, B], F32, name="sd")
            nc.scalar.activation(out=sd[:], in_=vsum[:], func=AF.Sqrt,
                                 scale=1.0 / F, bias=eps_col[:, :1])
            rstd = epool.tile([128, B], F32, name="rstd")
            nc.vector.reciprocal(rstd[:], sd[:])
            if xh is None:
                xh = epool.tile([128, B, F], F32, name="xh")
                nc.vector.tensor_tensor(
                    out=xh[:], in0=xc[:],
                    in1=rstd[:, :, None].to_broadcast([128, B, F]),
                    op=OP.mult)
            if gammas[li] is not None:
                nc.vector.tensor_tensor(
                    out=xh[:], in0=xh[:],
                    in1=gammas[li][:, None, :].to_broadcast([128, B, F]),
                    op=OP.mult)
                nc.vector.tensor_tensor(
                    out=xh[:], in0=xh[:],
                    in1=betas[li][:, None, :].to_broadcast([128, B, F]),
                    op=OP.add)
            # ELU = (max(x,0)-1) + exp(min(x,0))
            mm_ = epool.tile([128, B, F], F32, name="elum")
            nc.vector.tensor_scalar_min(mm_[:], xh[:], 0.0)
            ee = epool.tile([128, B, F], F32, name="elue")
            nc.scalar.activation(out=ee[:], in_=mm_[:], func=AF.Exp)
            rl = epool.tile([128, B, F], F32, name="elur")
            nc.vector.tensor_scalar(rl[:], xh[:], 0.0, -1.0, OP.max, OP.add)
            h = epool.tile([128, B, F], BF16 if out_bf else F32, name="eluh")
            nc.vector.tensor_tensor(out=h[:], in0=rl[:], in1=ee[:], op=OP.add)
            hd = None
            if hd_out:
                hd = epool.tile([128, B, F], BF16, name="hd")
                nc.vector.tensor_tensor(
                    out=hd[:], in0=h[:],
                    in1=dinv_b[:, :, None].to_broadcast([128, B, F]),
                    op=OP.mult)
            return h, hd

        def conv(li, table_t, F, wnext, fnext, bounce_t):
            """One GCN conv layer. li: 0/1/2. F: table feature width.
            wnext/fnext: weight tile + out width for next table (or None
            for conv3 -> final layers)."""
            # own-shard rows resident for the self-loop contribution:
            # own[p, w, :] = table[CORE*SLOTS + w*128 + p]. Loaded via the
            # partition-id-relative slice: each core reads ITS block.
            own = opool.tile([128, NWIN, 128], BF16, name="own")
            nc.sync.dma_start(
                out=own[:, :, :F],
                in_=own_src_t[li][:, :F].rearrange("(w p) f -> p w f", p=128))
            seg_i = 0
            for s in range(nsweep):
                w0 = s * sweep_w
                nw = min(sweep_w, NWIN - w0)
                swt = pp_sweep.tile([128, nw * F], F32, name="sweep")
                # zero-clear via K=1 matmuls (512-f32 spans)
                if not DBG_NOZERO:
                    for b0 in range(0, nw * F, 512):
                        span = min(512, nw * F - b0)
                        nc.tensor.matmul(out=swt[:, b0:b0 + span], lhsT=zw[:1, :],
                                         rhs=zr[:1, :span], start=True, stop=False,
                                         skip_group_check=True)
                # chunks of this sweep, per r segment
                while seg_i < len(segs) and segs[seg_i][0] == s:
                    _, r, ks = segs[seg_i]
                    seg_i += 1
                    for c0 in range(0, len(ks), CALL_CHUNKS):
                        kk = ks[c0:c0 + CALL_CHUNKS]
                        ncall = len(kk)
                        p0 = kk[0] * CHUNK
                        it = mpool.tile([128, CALL_CHUNKS * 8], I16, name="idx")
                        nc.sync.dma_start(
                            out=it[:, :ncall * 8],
                            in_=gidx[:, p0 // 16:(p0 + ncall * CHUNK) // 16])
                        qn = call_counter[0] % NQUEUES
                        call_counter[0] += 1
                        msg = mpool.tile([128, CALL_CHUNKS, 128], BF16, name="msg")
                        if DBG_NOGATHER:
                            nc.vector.memset(msg[:, :ncall, :], 0.0)
                        else:
                            nc.gpsimd.dma_gather(
                                out_ap=msg[:, :ncall, :],
                                in_ap=table_t[r * RS:(r + 1) * RS, :],
                                idxs_ap=it[:, :ncall * 8],
                                num_idxs=ncall * CHUNK,
                                num_idxs_reg=ncall * CHUNK,
                                elem_size=128,
                                queue_num=qn,
                            )
                        for b0 in range(0, ncall, SBATCH):
                            nb = min(SBATCH, ncall - b0)
                            St = spool.tile([128, SBATCH, 128], BF16, name="S")
                            k0 = kk[b0]
                            if DBG_NOS:
                                nc.vector.memset(St[:, :nb, :], 0.0)
                            elif True:
                                nc.vector.tensor_tensor(
                                out=St[:, :nb, :],
                                in0=dstloc[:, k0:k0 + nb, None].to_broadcast(
                                    [128, nb, 128]),
                                in1=iota[:, None, :].to_broadcast([128, nb, 128]),
                                op=OP.is_equal)
                            for j in range(nb):
                                if DBG_NOMM:
                                    continue
                                k = kk[b0 + j]
                                w = meta["pos_meta"][k][0]
                                uo = (w - w0) * F
                                nc.tensor.matmul(
                                    out=swt[:, uo:uo + F],
                                    lhsT=St[:, j, :],
                                    rhs=msg[:, b0 + j, :F],
                                    start=False, stop=False,
                                    skip_group_check=True)
                # epilogue for this sweep
                if int(__import__("os").environ.get("NOEPI", "0")):
                    continue
                for u0 in range(0, nw, EPB):
                    B = min(EPB, nw - u0)
                    dv = dinvu[:, w0 + u0:w0 + u0 + B]
                    own_sl = own[:, w0 + u0:w0 + u0 + B, :F]
                    if wnext is not None:
                        h, hd = ln_elu_batch(swt[:, u0 * F:(u0 + B) * F].rearrange(
                            "p (b f) -> p b f", b=B), B, F, li, dv,
                            out_bf=False, hd_out=True, own_ap=own_sl)
                        spl = spool.tile([128, EPB, 128], BF16, name="spl")
                        for u in range(B):
                            aux = pp_aux.tile([128, 128], BF16, name="aux")
                            nc.tensor.transpose(out=aux[:F, :], in_=hd[:, u, :],
                                                identity=ident[:])
                            hdT = spool.tile([F, 128], BF16, name="hdT")
                            nc.scalar.copy(out=hdT[:], in_=aux[:F, :])
                            mmp = pp_mm.tile([128, 128], F32, name="mmp")
                            nc.tensor.matmul(out=mmp[:, :fnext], lhsT=hdT[:],
                                             rhs=wnext[:], start=True, stop=True,
                                             skip_group_check=True)
                            nc.scalar.copy(out=spl[:, u, :fnext],
                                           in_=mmp[:, :fnext])
                        ug0 = w0 + u0
                        nc.sync.dma_start(
                            out=bounce_t[ug0 * 128:(ug0 + B) * 128, :fnext]
                            .rearrange("(b p) f -> p b f", p=128),
                            in_=spl[:, :B, :fnext])
                    else:
                        # conv3: final layers
                        h, _ = ln_elu_batch(swt[:, u0 * F:(u0 + B) * F].rearrange(
                            "p (b f) -> p b f", b=B), B, F, li, dv,
                            out_bf=True, hd_out=False, own_ap=own_sl)
                        f1 = pp_f1.tile([128, EPB * 32], F32, name="f1")
                        for u in range(B):
                            aux = pp_aux.tile([128, 128], BF16, name="aux")
                            nc.tensor.transpose(out=aux[:64, :], in_=h[:, u, :],
                                                identity=ident[:])
                            h3T = spool.tile([64, 128], BF16, name="hdT")
                            nc.scalar.copy(out=h3T[:], in_=aux[:64, :])
                            nc.tensor.matmul(out=f1[:, u * 32:(u + 1) * 32],
                                             lhsT=h3T[:], rhs=lw1[:],
                                             start=True, stop=True,
                                             skip_group_check=True)
                        y1, _ = ln_elu_batch(f1[:, :B * 32].rearrange(
                            "p (b f) -> p b f", b=B), B, 32, 3, None,
                            out_bf=True, hd_out=False)
                        ys = epool.tile([128, EPB, 32], F32, name="ys")
                        for u in range(B):
                            aux = pp_aux.tile([128, 128], BF16, name="aux")
                            nc.tensor.transpose(out=aux[:32, :], in_=y1[:, u, :],
                                                identity=ident[:])
                            y1T = spool.tile([32, 128], BF16, name="y1T")
                            nc.scalar.copy(out=y1T[:], in_=aux[:32, :])
                            f2 = pp_f2.tile([128, 32], F32, name="f2")
                            nc.tensor.matmul(out=f2[:], lhsT=y1T[:], rhs=lw2[:],
                                             start=True, stop=True,
                                             skip_group_check=True)
                            nc.vector.tensor_tensor(
                                out=ys[:, u, :], in0=f2[:],
                                in1=biases[4][:, :],
                                op=OP.add)
                        ug0 = w0 + u0
                        nc.sync.dma_start(
                            out=out_y[ug0 * 128:(ug0 + B) * 128, :]
                            .rearrange("(b p) f -> p b f", p=128),
                            in_=ys[:, :B, :])

        stage = DBG_STAGE
        call_counter = [0]
        own_src_t = {0: own1_d, 1: bounce2, 2: bounce3}
        for _rep in range(REPEAT):
            conv(0, table1, 64, w2, 128, bounce2)
            if stage >= 2:
                if not DBG_NOCOLL:
                    nc.gpsimd.collective_compute(
                        "AllGather", OP.bypass, replica_groups=[list(range(8))],
                        ins=[bounce2[:, :]], outs=[table2[:, :]])
                conv(1, table2, 128, w3, 64, bounce3)
            if stage >= 3:
                if not DBG_NOCOLL:
                    nc.gpsimd.collective_compute(
                        "AllGather", OP.bypass, replica_groups=[list(range(8))],
                        ins=[bounce3[:, :]], outs=[table3[:, :]])
                conv(2, table3, 64, None, None, None)
        if stage < 3:
            # dummy writes so out_y is produced
            zo = cpool.tile([128, 32], F32)
            nc.vector.memset(zo[:], 0.0)
            for w in range(NWIN):
                nc.sync.dma_start(out=out_y[w * 128:(w + 1) * 128, :], in_=zo[:])

    nc.finalize()
    return nc


def make_inputs(meta, x, W1, b1, W2, b2, W3, b3, lw1, lb1, lw2, lb2,
                gs, bes):
    """Per-core input arrays. gs/bes: [g1,g2,g3,g4], [be1..be4]."""
    N = x.shape[0]
    NS, SLOTS, NWIN = meta["NS"], meta["SLOTS"], meta["NWIN"]
    perm = meta["perm_rows"]
    dinv = meta["dinv"]

    xs1 = (x.astype(np.float32) @ W1.astype(np.float32)) * dinv[:, None]
    t1 = np.zeros((8 * SLOTS, 128), ml_dtypes.bfloat16)
    t1[perm, :64] = xs1.astype(ml_dtypes.bfloat16)

    iota = np.tile(np.arange(128, dtype=np.float32), (128, 1)).astype(
        ml_dtypes.bfloat16)

    ln_nontrivial = [not (np.abs(g - 1).max() < 1e-12 and
                          np.abs(b).max() < 1e-12)
                     for g, b in zip(gs, bes)]

    in_maps = []
    for c in range(8):
        # dinv per (partition, window): node of slot (w,p)
        dinvu = np.zeros((128, NWIN), np.float32)
        node_ids = c * NS + np.arange(NS)
        rows = perm[node_ids] - c * SLOTS
        dinvu[rows % 128, rows // 128] = dinv[node_ids]
        gidx = meta["g_idx"][c]
        gidx_w = np.tile(gidx.reshape(-1, 16).T, (8, 1)).copy()
        dstloc = meta["g_dstloc"][c].reshape(-1, 128).T.astype(
            ml_dtypes.bfloat16).copy()
        d = dict(
            table1=t1, own1=t1[c * SLOTS:(c + 1) * SLOTS].copy(),
            gidx=gidx_w, dstloc=dstloc, iota=iota,
            dinvu=dinvu,
            bias0=np.tile(b1.reshape(1, -1), (128, 1)).astype(np.float32),
            bias1=np.tile(b2.reshape(1, -1), (128, 1)).astype(np.float32),
            bias2=np.tile(b3.reshape(1, -1), (128, 1)).astype(np.float32),
            bias3=np.tile(lb1.reshape(1, -1), (128, 1)).astype(np.float32),
            bias4=np.tile(lb2.reshape(1, -1), (128, 1)).astype(np.float32),
            w2b=W2.astype(ml_dtypes.bfloat16),
            w3b=W3.astype(ml_dtypes.bfloat16),
            lw1b=lw1.astype(ml_dtypes.bfloat16),
            lw2b=lw2.astype(ml_dtypes.bfloat16),
        )
        for i in range(4):
            if ln_nontrivial[i]:
                d[f"gam{i}"] = np.tile(gs[i].reshape(1, -1), (128, 1)).astype(np.float32)
                d[f"bet{i}"] = np.tile(bes[i].reshape(1, -1), (128, 1)).astype(np.float32)
        in_maps.append(d)
    return in_maps, ln_nontrivial


def unshard_output(meta, results):
    """results: list of 8 dicts with out_y [SLOTS, 32]. Returns [N, 32]."""
    NS, SLOTS = meta["NS"], meta["SLOTS"]
    N = 8 * NS
    out = np.zeros((N, 32), np.float32)
    for c in range(8):
        rows = meta["perm_rows"][c * NS:(c + 1) * NS] - c * SLOTS
        out[c * NS:(c + 1) * NS] = results[c]["out_y"][rows]
    return out


class BassRunner:
    def __init__(self, nc, n_cores=8):
        install_neuronx_cc_hook()
        self.nc = nc
        self.n_cores = n_cores
        part_name = (nc.partition_id_tensor.name
                     if nc.partition_id_tensor else None)
        in_names, out_names, out_avals, zero_outs = [], [], [], []
        for alloc in nc.m.functions[0].allocations:
            if not isinstance(alloc, mybir.MemoryLocationSet):
                continue
            name = alloc.memorylocations[0].name
            if alloc.kind == "ExternalInput":
                if name != part_name:
                    in_names.append(name)
            elif alloc.kind == "ExternalOutput":
                shape = tuple(alloc.tensor_shape)
                dtype = mybir.dt.np(alloc.dtype)
                out_names.append(name)
                out_avals.append(jax.core.ShapedArray(shape, dtype))
                zero_outs.append(np.zeros(shape, dtype))
        self.in_names = list(in_names)
        self.out_names = out_names
        self.zero_outs = zero_outs
        n_params = len(in_names)
        n_outs = len(out_avals)
        all_in_names = in_names + out_names
        if part_name is not None:
            all_in_names = all_in_names + [part_name]
        donate = tuple(range(n_params, n_params + n_outs))
        self.n_params = n_params

        def _body(*args):
            operands = list(args)
            if part_name is not None:
                operands.append(bass2jax.partition_id_tensor())
            outs = _bass_exec_p.bind(
                *operands, out_avals=tuple(out_avals),
                in_names=tuple(all_in_names), out_names=tuple(out_names),
                lowering_input_output_aliases=(),
                sim_require_finite=True, sim_require_nnan=True, nc=nc)
            return tuple(outs)

        devices = jax.devices()[:n_cores]
        self.mesh = Mesh(np.asarray(devices), ("core",))
        in_specs = (PartitionSpec("core"),) * (n_params + n_outs)
        out_specs = (PartitionSpec("core"),) * len(out_names)
        self.fn = jax.jit(
            shard_map(_body, mesh=self.mesh, in_specs=in_specs,
                      out_specs=out_specs, check_rep=False),
            donate_argnums=donate, keep_unused=True)
        self.sharding = NamedSharding(self.mesh, PartitionSpec("core"))

    def put_inputs(self, in_maps):
        concat = [np.concatenate([np.asarray(in_maps[c][n])
                                  for c in range(self.n_cores)], axis=0)
                  for n in self.in_names]
        self.in_dev = [jax.device_put(a, self.sharding) for a in concat]

    def _zeros_dev(self):
        return [jax.device_put(
            np.zeros((self.n_cores * z.shape[0], *z.shape[1:]), z.dtype),
            self.sharding) for z in self.zero_outs]

    def run(self):
        outs = self.fn(*self.in_dev, *self._zeros_dev())
        jax.block_until_ready(outs)
        return outs

    def run_timed(self, iters=3):
        """Returns (best_seconds, outs)."""
        zeros = [self._zeros_dev() for _ in range(iters)]
        best = float("inf")
        outs = None
        for i in range(iters):
            t0 = time.perf_counter()
            outs = self.fn(*self.in_dev, *zeros[i])
            jax.block_until_ready(outs)
            dt = time.perf_counter() - t0
            best = min(best, dt)
        return best, outs

    def run_pipelined(self, iters=8):
        """Submit `iters` executions without intermediate blocking; returns
        total wall seconds. Async dispatch means wall ~= const + iters*exec
        if the device queue pipelines."""
        zeros = [self._zeros_dev() for _ in range(iters)]
        outs = None
        t0 = time.perf_counter()
        for i in range(iters):
            outs = self.fn(*self.in_dev, *zeros[i])
        jax.block_until_ready(outs)
        return time.perf_counter() - t0

    def results(self, outs):
        res = []
        for c in range(self.n_cores):
            d = {}
            for i, n in enumerate(self.out_names):
                arr = np.asarray(outs[i])
                per = arr.shape[0] // self.n_cores
                d[n] = arr[c * per:(c + 1) * per]
            res.append(d)
        return res


_CACHE = {}


def kernel(**inputs):
    x = np.asarray(inputs["x"], np.float32)
    ei = np.asarray(inputs["edge_index"], np.int64)
    gs = [np.asarray(inputs[k], np.float32) for k in ("g1", "g2", "g3", "g4")]
    bes = [np.asarray(inputs[k], np.float32) for k in ("be1", "be2", "be3", "be4")]
    meta = prep(ei, N_NODES, sweep_w=8)
    in_maps, ln_nt = make_inputs(
        meta, x, inputs["W1"], inputs["b1"], inputs["W2"], inputs["b2"],
        inputs["W3"], inputs["b3"], inputs["lw1"], inputs["lb1"],
        inputs["lw2"], inputs["lb2"], gs, bes)
    key = (meta["npos"], tuple(ln_nt))
    if key not in _CACHE:
        nc = build(meta, dict(ln_nontrivial=ln_nt + [False]))
        r = BassRunner(nc, NCORES)
        _CACHE[key] = r
    r = _CACHE[key]
    r.put_inputs(in_maps)
    outs = r.run()
    res = r.results(outs)
    out = unshard_output(meta, res)
    # expose for test harness timing
    kernel._last_runner = r
    kernel._last_meta = meta
    return out

